# revision 2
# baseline (speedup 1.0000x reference)
"""GATv2 (2-layer + linear head) Trainium2 Bass kernel, 8-core SPMD.

Strategy: edges sorted by dst, dst-range-partitioned across 8 cores.  Per
core, edges are processed in batches of 4 windows of 128 dst nodes; per
128-edge group a weighted one-hot matrix scatters [exp(e)*xl[src] | exp(e)]
into a PSUM accumulator via one tensor-engine matmul (the table carries a
constant-1 column so numerator and denominator share the matmul); segment
softmax uses unshifted exp (shift-invariant, safe range here).  Node-level
linears run data-parallel on device into gather tables; xl gathers use
int16 dma_gather with 32768-row chunking; xr gathers read a per-device
dst-slice table at device-independent addresses (SPMD: one instruction
stream for all cores, plan structure padded to cross-device maxima).
"""
import sys
sys.path.insert(0, '/opt/trn_rl_repo')
import numpy as np

P = 128
N = 100000
F = 128
H1 = 64
H2 = 32
NDEV = 8
DN = N // NDEV            # 12500 dst nodes per device
CHUNK = 32768             # int16 gather index limit
NW = (DN + P - 1) // P    # 98 windows per device
BW = 4                    # windows per batch
NCHUNK = (N + CHUNK - 1) // CHUNK  # 4
MAXG = 8                  # max 128-edge groups per dma_gather (1024-desc ring)
DNP = NW * P              # 12544 padded dst rows
ROWL1 = 128               # tabL row floats, layer1: [xl(64) | 1 | pad]
ROWL2 = 64                # tabL row floats, layer2: [xl2(32) | 1 | pad]
NB = 8                    # node tiles per batched DMA


def _batches():
    out = []
    w = 0
    while w < NW:
        k = min(BW, NW - w)
        out.append((w, k))
        w += k
    return out


def _pack_idx16(idx):
    """idx: int array, len multiple of 128 -> [128, len//16] int16 tile data.
    Logical position i lives at [i % 16, i // 16], replicated over the 8
    16-partition groups (each SWDGE queue's Q7 pair reads its own group)."""
    n = len(idx)
    a = np.asarray(idx, np.int16).reshape(n // 16, 16).T  # [16, n//16]
    return np.tile(a, (8, 1))


def _build_plan(src, dst):
    """src/dst: int64 (dst-sorted, len E_tot).  Uniform instruction structure
    across cores: per (batch, src-chunk, window-in-batch) group counts are
    maxima over cores."""
    batches = _batches()
    NBT = len(batches)
    counts = np.zeros((NDEV, NBT, NCHUNK, BW), np.int64)
    w2b = np.zeros(NW, np.int64)
    w2i = np.zeros(NW, np.int64)
    for bi, (w0, k) in enumerate(batches):
        w2b[w0:w0 + k] = bi
        w2i[w0:w0 + k] = np.arange(k)
    dev_edges = []
    for d in range(NDEV):
        lo, hi = np.searchsorted(dst, [DN * d, DN * (d + 1)])
        s = src[lo:hi]
        t = dst[lo:hi] - DN * d
        ck = s // CHUNK
        wloc = t // P
        bi = w2b[wloc]
        wi = w2i[wloc]
        key = (bi * NCHUNK + ck) * BW + wi
        np.add.at(counts[d].reshape(-1), key, 1)
        order = np.lexsort((t, ck, bi))
        dev_edges.append((s[order], t[order], key[order]))

    gu = (counts.max(axis=0) + P - 1) // P       # [NBT, NCHUNK, BW]
    layout = []
    icol_off = 0
    gcol_off = 0
    for bi, (w0, k) in enumerate(batches):
        gp = 0
        instrs = []        # (chunk, group offset in batch, G)
        groups_w = []      # window-in-batch per group
        for c in range(NCHUNK):
            run = 0
            for w in range(BW):
                run += gu[bi, c, w]
                groups_w += [w] * int(gu[bi, c, w])
            a = 0
            while a < run:
                g = min(MAXG, run - a)
                instrs.append((c, gp + a, g))
                a += g
            gp += run
        icols = sum(16 * g for (_, _, g) in instrs)
        layout.append(dict(bi=bi, w0=w0, nw=k, GP=gp, instrs=instrs,
                           groups_w=groups_w, icol_off=icol_off,
                           gcol_off=gcol_off))
        icol_off += icols
        gcol_off += gp
    ICT, GCT = icol_off, gcol_off

    idx_all = np.zeros((NDEV, 128, ICT), np.int16)
    dstl_all = np.full((NDEV, 128, GCT), -1.0, np.float32)
    gu_flat = gu.reshape(-1)
    base_of_key = np.zeros(gu_flat.size + 1, np.int64)
    base_of_key[1:] = np.cumsum(gu_flat * P)
    gtot = int(gu.sum())
    for d in range(NDEV):
        s, t, key = dev_edges[d]
        kchange = np.r_[True, key[1:] != key[:-1]]
        runstart = np.maximum.accumulate(
            np.where(kchange, np.arange(len(key)), 0))
        within = np.arange(len(key)) - runstart
        slot = base_of_key[key] + within
        E_pad = gtot * P
        xl_rel = np.zeros(E_pad, np.int64)
        xr_rel = np.zeros(E_pad, np.int64)
        dstl_v = np.full(E_pad, -1.0, np.float32)
        xl_rel[slot] = s - (s // CHUNK) * CHUNK
        wloc = t // P
        xr_rel[slot] = t - np.array([b[0] for b in batches])[w2b[wloc]] * P
        dstl_v[slot] = (t - wloc * P).astype(np.float32)
        for L in layout:
            bi, gp = L["bi"], L["GP"]
            e0 = base_of_key[(bi * NCHUNK) * BW]
            dv = dstl_v[e0:e0 + gp * P].reshape(gp, P).T
            dstl_all[d, :, L["gcol_off"]:L["gcol_off"] + gp] = dv
            ic = L["icol_off"]
            for (c, goff, G) in L["instrs"]:
                a0 = e0 + goff * P
                a1 = a0 + G * P
                idx_all[d, :, ic:ic + 8 * G] = _pack_idx16(xl_rel[a0:a1])
                idx_all[d, :, ic + 8 * G:ic + 16 * G] = _pack_idx16(xr_rel[a0:a1])
                ic += 16 * G
    return layout, ICT, GCT, idx_all, dstl_all


def _emit_node_pass(nc, tc, npool, npsum, mybir, AL, src_dram, wc, bias_bc,
                    dst_dram, nrows, Cin, ncols):
    """Batched x @ W + b -> table.  src_dram [Cin, >=nrows] (transposed),
    dst_dram [>=nrows, rowW]; writes cols [0:ncols]."""
    f32 = mybir.dt.float32
    nt = (nrows + P - 1) // P
    blk = 0
    while blk < nt:
        k = min(NB, nt - blk)
        r0 = blk * P
        rows = min(nrows - r0, k * P)
        full = (rows == k * P)
        xt = npool.tile([Cin, NB * P], f32, tag="xt", name="xt")
        nc.sync.dma_start(out=xt[:, :rows], in_=src_dram[:, r0:r0 + rows])
        ot = npool.tile([P, NB, ncols], f32, tag="ot", name="ot")
        for i in range(k):
            nv = min(P, rows - i * P)
            ps = npsum.tile([P, ncols], f32, space="PSUM", tag="ps", name="ps")
            nc.tensor.matmul(out=ps[:nv, :], lhsT=xt[:, i * P:i * P + nv],
                             rhs=wc[:], start=True, stop=True)
            nc.vector.tensor_tensor(out=ot[:nv, i, :], in0=ps[:nv, :],
                                    in1=bias_bc[:nv, :], op=AL.add)
            if not full:
                nc.sync.dma_start(
                    out=dst_dram[r0 + i * P:r0 + i * P + nv, 0:ncols],
                    in_=ot[:nv, i, :])
        if full:
            dv = dst_dram[r0:r0 + k * P, 0:ncols].rearrange(
                "(b p) c -> p b c", p=P)
            nc.sync.dma_start(out=dv, in_=ot[:, :k, :])
        blk += k


def _build_gat_layer(Cin, Cout, layout, ICT, GCT, final_linear):
    """One dispatch: node-phase linears into gather tables, then the edge
    phase (gather + segment softmax + one-hot scatter matmuls)."""
    import concourse.bacc as bacc
    import concourse.bass as bass
    import concourse.mybir as mybir
    import concourse.tile as tile
    from concourse import library_config

    f32 = mybir.dt.float32
    i16 = mybir.dt.int16
    AL = mybir.AluOpType
    ROW = ROWL1 if Cout == H1 else ROWL2

    nc = bacc.Bacc("TRN2", target_bir_lowering=False, debug=False,
                   num_swdge_queues=4)
    t_xT = nc.dram_tensor("xT", [Cin, N], f32, kind="ExternalInput")
    t_xdT = nc.dram_tensor("xdT", [Cin, DNP], f32, kind="ExternalInput")
    t_wl = nc.dram_tensor("wl", [Cin, Cout + 1], f32, kind="ExternalInput")
    t_wr = nc.dram_tensor("wr", [Cin, Cout], f32, kind="ExternalInput")
    t_bl = nc.dram_tensor("bl", [128, Cout + 1], f32, kind="ExternalInput")
    t_br = nc.dram_tensor("br", [128, Cout], f32, kind="ExternalInput")
    t_attb = nc.dram_tensor("attb", [128, MAXG * Cout], f32, kind="ExternalInput")
    if final_linear:
        t_wlinb = nc.dram_tensor("wlinb", [128, Cout], f32, kind="ExternalInput")
        t_blin2 = nc.dram_tensor("blin2", [128, 1], f32, kind="ExternalInput")
        t_out = nc.dram_tensor("out", [DNP, 1], f32, kind="ExternalOutput")
        OC = 1
    else:
        t_b1o = nc.dram_tensor("b1o", [128, Cout], f32, kind="ExternalInput")
        t_out = nc.dram_tensor("h", [DNP, Cout], f32, kind="ExternalOutput")
        OC = Cout
    t_eidx = nc.dram_tensor("eidx", [128, ICT], i16, kind="ExternalInput")
    t_dstl = nc.dram_tensor("dstl", [128, GCT], f32, kind="ExternalInput")
    tabL = nc.dram_tensor("tabL", [(N + P - 1) // P * P, ROW], f32, kind="Internal")
    tabR = nc.dram_tensor("tabR", [DNP, 64], f32, kind="Internal")

    with tile.TileContext(nc) as tc:
        nc.gpsimd.load_library(library_config.mlp)
        with tc.tile_pool(name="const", bufs=1) as cpool:
            iota = cpool.tile([P, P], f32)
            nc.gpsimd.iota(iota[:], pattern=[[1, P]], base=0, channel_multiplier=0,
                           allow_small_or_imprecise_dtypes=True)
            attb = cpool.tile([P, MAXG * Cout], f32)
            nc.sync.dma_start(out=attb[:], in_=t_attb[:])
            wl = cpool.tile([Cin, Cout + 1], f32)
            wr = cpool.tile([Cin, Cout], f32)
            bl = cpool.tile([P, Cout + 1], f32)
            br = cpool.tile([P, Cout], f32)
            nc.sync.dma_start(out=wl[:], in_=t_wl[:])
            nc.sync.dma_start(out=wr[:], in_=t_wr[:])
            nc.sync.dma_start(out=bl[:], in_=t_bl[:])
            nc.sync.dma_start(out=br[:], in_=t_br[:])
            if final_linear:
                wlinb = cpool.tile([P, Cout], f32)
                nc.sync.dma_start(out=wlinb[:], in_=t_wlinb[:])
                blin2 = cpool.tile([P, 1], f32)
                nc.sync.dma_start(out=blin2[:], in_=t_blin2[:])
            else:
                b1o = cpool.tile([P, Cout], f32)
                nc.sync.dma_start(out=b1o[:], in_=t_b1o[:])

            # ---------------- node phase ----------------
            with tc.tile_pool(name="nsb", bufs=4) as npool, \
                 tc.tile_pool(name="nps", bufs=4, space="PSUM") as npsum:
                _emit_node_pass(nc, tc, npool, npsum, mybir, AL,
                                t_xT, wl, bl, tabL, N, Cin, Cout + 1)
                _emit_node_pass(nc, tc, npool, npsum, mybir, AL,
                                t_xdT, wr, br, tabR, DNP, Cin, Cout)

            tc.strict_bb_all_engine_barrier()

            # ---------------- edge phase ----------------
            with tc.tile_pool(name="esb", bufs=2) as ep, \
                 tc.tile_pool(name="exl", bufs=6) as xp, \
                 tc.tile_pool(name="etmp", bufs=4) as tp, \
                 tc.tile_pool(name="eps", bufs=8, space="PSUM") as eps:
                qn = 0
                for L in layout:
                    w0, nw, GP = L["w0"], L["nw"], L["GP"]
                    icols = sum(16 * g for (_, _, g) in L["instrs"])
                    idxT = ep.tile([P, icols], i16, tag="idx", name="idx")
                    nc.sync.dma_start(
                        out=idxT[:],
                        in_=t_eidx[:, L["icol_off"]:L["icol_off"] + icols])
                    dstlT = ep.tile([P, GP], f32, tag="dstl", name="dstl")
                    nc.sync.dma_start(
                        out=dstlT[:],
                        in_=t_dstl[:, L["gcol_off"]:L["gcol_off"] + GP])
                    eT = ep.tile([P, GP], f32, tag="e", name="e")
                    wT = ep.tile([P, GP], f32, tag="w", name="w")
                    acc = [eps.tile([P, Cout + 1], f32, space="PSUM", tag="acc",
                                    name=f"acc{i}") for i in range(nw)]
                    gw = L["groups_w"]
                    first = [True] * nw
                    lastg = [max((g for g in range(GP) if gw[g] == w), default=-1)
                             for w in range(nw)]
                    outt = ep.tile([P, BW, OC], f32, tag="outt", name="outt")

                    ic = 0
                    for (c, goff, G) in L["instrs"]:
                        xl = xp.tile([P, MAXG, ROW], f32, tag="xl", name="xl")
                        xr = tp.tile([P, MAXG, 64], f32, tag="xr", name="xr")
                        ni = G * P
                        nc.gpsimd.dma_gather(
                            xl[:, :G, :], tabL[c * CHUNK:, :],
                            idxT[:, ic:ic + 8 * G], ni, ni, ROW,
                            queue_num=qn)
                        nc.gpsimd.dma_gather(
                            xr[:, :G, :], tabR[w0 * P:, :],
                            idxT[:, ic + 8 * G:ic + 16 * G], ni, ni, 64,
                            queue_num=(qn + 1) % 4)
                        qn = (qn + 2) % 4
                        ic += 16 * G
                        z = tp.tile([P, MAXG * Cout], f32, tag="z", name="z")
                        zs = tp.tile([P, MAXG * Cout], f32, tag="zs", name="zs")
                        z3 = z[:, :G * Cout].rearrange("p (g c) -> p g c", g=G)
                        zs3 = zs[:, :G * Cout].rearrange("p (g c) -> p g c", g=G)
                        nc.vector.tensor_tensor(out=z3, in0=xl[:, :G, 0:Cout],
                                                in1=xr[:, :G, 0:Cout], op=AL.add)
                        nc.scalar.mul(zs3, z3, 0.2)
                        nc.vector.tensor_tensor(out=z3, in0=zs3, in1=z3, op=AL.max)
                        nc.vector.tensor_tensor(
                            out=z3, in0=z3,
                            in1=attb[:, :G * Cout].rearrange("p (g c) -> p g c", g=G),
                            op=AL.mult)
                        nc.vector.tensor_reduce(out=eT[:, goff:goff + G], in_=z3,
                                                axis=mybir.AxisListType.X, op=AL.add)
                        nc.scalar.activation(out=wT[:, goff:goff + G],
                                             in_=eT[:, goff:goff + G],
                                             func=mybir.ActivationFunctionType.Exp)
                        for gi in range(G):
                            g = goff + gi
                            w = gw[g]
                            B = tp.tile([P, P], f32, tag="B", name="B")
                            nc.vector.tensor_scalar(out=B[:], in0=iota[:],
                                                    scalar1=dstlT[:, g:g + 1],
                                                    scalar2=wT[:, g:g + 1],
                                                    op0=AL.is_equal, op1=AL.mult)
                            nc.tensor.matmul(out=acc[w][:], lhsT=B[:],
                                             rhs=xl[:, gi, 0:Cout + 1],
                                             start=first[w],
                                             stop=(g == lastg[w]))
                            first[w] = False

                    for w in range(nw):
                        r = tp.tile([P, 1], f32, tag="r", name="r")
                        nc.vector.reciprocal(r[:], acc[w][:, Cout:Cout + 1])
                        if final_linear:
                            v = tp.tile([P, Cout], f32, tag="v", name="v")
                            nc.vector.tensor_tensor(out=v[:], in0=acc[w][:, :Cout],
                                                    in1=wlinb[:], op=AL.mult)
                            sv = tp.tile([P, 1], f32, tag="sv", name="sv")
                            nc.vector.tensor_reduce(out=sv[:], in_=v[:],
                                                    axis=mybir.AxisListType.X,
                                                    op=AL.add)
                            sv2 = tp.tile([P, 1], f32, tag="sv2", name="sv2")
                            nc.vector.tensor_scalar(out=sv2[:], in0=sv[:],
                                                    scalar1=r[:], scalar2=None,
                                                    op0=AL.mult)
                            nc.vector.tensor_tensor(out=outt[:, w, :], in0=sv2[:],
                                                    in1=blin2[:], op=AL.add)
                        else:
                            t1 = tp.tile([P, Cout], f32, tag="t1", name="t1")
                            nc.vector.tensor_scalar(out=t1[:], in0=acc[w][:, :Cout],
                                                    scalar1=r[:], scalar2=None,
                                                    op0=AL.mult)
                            t2 = tp.tile([P, Cout], f32, tag="t2", name="t2")
                            nc.vector.tensor_tensor(out=t2[:], in0=t1[:],
                                                    in1=b1o[:], op=AL.add)
                            t3 = tp.tile([P, Cout], f32, tag="t3", name="t3")
                            nc.vector.tensor_scalar(out=t3[:], in0=t2[:],
                                                    scalar1=0.01, scalar2=None,
                                                    op0=AL.mult)
                            nc.vector.tensor_tensor(out=outt[:, w, :], in0=t3[:],
                                                    in1=t2[:], op=AL.max)
                    ov = t_out[w0 * P:(w0 + nw) * P, :].rearrange(
                        "(b p) c -> p b c", p=P)
                    nc.sync.dma_start(out=ov, in_=outt[:, :nw, :])
    nc.compile()
    return nc


_CACHE = {}


def kernel(x, edge_index, W1l, b1l, W1r, b1r, att1, bias1,
           W2l, b2l, W2r, b2r, att2, bias2, Wlin, blin):
    from concourse import bass_utils

    x = np.asarray(x, np.float32)
    edge_index = np.asarray(edge_index)
    src = np.concatenate([edge_index[0], np.arange(N, dtype=edge_index.dtype)]).astype(np.int64)
    dst = np.concatenate([edge_index[1], np.arange(N, dtype=edge_index.dtype)]).astype(np.int64)
    order = np.argsort(dst, kind="stable")
    src, dst = src[order], dst[order]

    layout, ICT, GCT, idx_all, dstl_all = _build_plan(src, dst)

    def bcast(v, n=128):
        return np.tile(np.asarray(v, np.float32)[None, :], (n, 1))

    key = ("k", ICT, GCT)
    if key not in _CACHE:
        _CACHE[key] = (
            _build_gat_layer(F, H1, layout, ICT, GCT, final_linear=False),
            _build_gat_layer(H1, H2, layout, ICT, GCT, final_linear=True),
        )
    ncA, ncB = _CACHE[key]

    # ---- dispatch A (layer 1) ----
    xT = np.ascontiguousarray(x.T)
    wl1 = np.zeros((F, H1 + 1), np.float32); wl1[:, :H1] = W1l
    bl1 = np.zeros((128, H1 + 1), np.float32)
    bl1[:, :H1] = np.asarray(b1l, np.float32); bl1[:, H1] = 1.0
    attb1 = np.tile(np.asarray(att1, np.float32)[None, :], (128, MAXG))
    in_maps = []
    for d in range(NDEV):
        xd = np.zeros((F, DNP), np.float32)
        xd[:, :DN] = x[DN * d:DN * (d + 1)].T
        in_maps.append(dict(
            xT=xT, xdT=xd, wl=wl1, wr=np.asarray(W1r, np.float32),
            bl=bl1, br=bcast(b1r), attb=attb1, b1o=bcast(bias1),
            eidx=idx_all[d], dstl=dstl_all[d]))
    resA = bass_utils.run_bass_kernel_spmd(ncA, in_maps, core_ids=list(range(NDEV)))
    h1 = np.concatenate([resA.results[d]["h"][:DN] for d in range(NDEV)], axis=0)

    # ---- dispatch B (layer 2 + head) ----
    h1T = np.ascontiguousarray(h1.T)
    wl2 = np.zeros((H1, H2 + 1), np.float32); wl2[:, :H2] = W2l
    bl2 = np.zeros((128, H2 + 1), np.float32)
    bl2[:, :H2] = np.asarray(b2l, np.float32); bl2[:, H2] = 1.0
    attb2 = np.tile(np.asarray(att2, np.float32)[None, :], (128, MAXG))
    wlinb = np.tile(np.asarray(Wlin, np.float32).reshape(1, H2), (128, 1))
    blin2 = float(np.asarray(bias2, np.float32) @ np.asarray(Wlin, np.float32).reshape(H2)
                  + np.asarray(blin, np.float32)[0])
    blin2t = np.full((128, 1), blin2, np.float32)
    in_maps = []
    for d in range(NDEV):
        hd = np.zeros((H1, DNP), np.float32)
        hd[:, :DN] = h1[DN * d:DN * (d + 1)].T
        in_maps.append(dict(
            xT=h1T, xdT=hd, wl=wl2, wr=np.asarray(W2r, np.float32),
            bl=bl2, br=np.tile(np.asarray(b2r, np.float32)[None, :], (128, 1)),
            attb=attb2, wlinb=wlinb, blin2=blin2t,
            eidx=idx_all[d], dstl=dstl_all[d]))
    resB = bass_utils.run_bass_kernel_spmd(ncB, in_maps, core_ids=list(range(NDEV)))
    out = np.concatenate([resB.results[d]["out"][:DN, 0] for d in range(NDEV)], axis=0)

    kernel._last_exec_ns = (resA.exec_time_ns, resB.exec_time_ns)
    kernel._trace_dirs = (resA.instructions_and_trace, resB.instructions_and_trace)
    return out

